# revision 29
# baseline (speedup 1.0000x reference)
"""DenseGINEConv on 8 TRN2 NeuronCores (Bass/Tile) — tri-route fp8 edition.

Reference computation (B=4, N=512, F=64, H=128):
    msg  = leaky_relu(adj[b,i,j] * (x[b,i,f] + edge_attr[b,i,j,f]), 0.01)
    agg  = sum_i msg                         # (B, N, F) indexed by destination j
    out  = x + agg
    h    = leaky_relu(out @ W1 + b1) @ W2 + b2
    res  = where(mask[b,j], h, 0)

Key facts:
  * adj >= 0, so lrelu(adj*z) = adj*lrelu(z) = 0.01*adj*z + 0.99*adj*relu(z).
  * Kept destination nodes only (host j-compaction); core c = 2*b + h.
  * Edge ships as fp8e4 (rel-err ~8e-3 vs the 2e-2 gate). The cost model
    prices a DMA at its OUTPUT bytes, so a cast-up CCE DMA is priced bf16.
    Hence three routes for z = x + e, u = relu(z), each burning a different
    scarce resource:
      V1 (CCE):   bf16 z tile prefilled with broadcast x (DVE 4x copy), SWDGE
                  CCE add casts fp8->bf16 in the DMA (DMA pays 2B/elem, DVE
                  pays 0.26ns/elem for prefill + relu).
      V2 (engine): plain fp8 e tile over HWDGE (1B/elem). Pool casts most
                  columns fp8->bf16, DVE adds broadcast x (2x) and a few
                  fully-fused columns (1x), relu via DVE tensor_scalar (4x).
      V3 (PE):    plain fp8 e tile. PE accumulates z in PSUM: an x-fill
                  matmul (xT stationary x broadcast-identity moving) plus an
                  e-copy matmul (fp8 identity stationary x e moving); ACT
                  (mostly) applies relu PSUM->SBUF bf16.
    The raw 0.01*z stream for V2/V3 never materializes z: sum_i adj*z =
    (x^T @ 0.01adj) dense matmuls + matvecs directly on the fp8 e tiles
    (mixed fp8 stationary x bf16 moving works).
  * Aggregation: per destination j, single-column matmuls accumulate
    oT[:, j] into one [F, Jp] PSUM tile (raw stream vs 0.01*adj, relu
    stream vs 0.99*adj; only the 0.99 copy ships, DVE derives 0.01).
  * MLP tail in two column chunks: y = relu(o@W1+b1) @ 0.99W2
    + o @ 0.01(W1@W2) + b2', b2' = 0.01*(b1@W2) + b2 host-folded.
"""
import numpy as np
import ml_dtypes

import concourse.bacc as bacc
import concourse.mybir as mybir
import concourse.tile as tile
from concourse.bass_utils import run_bass_kernel_spmd

B, N, F, H = 4, 512, 64, 128
NEG_SLOPE = 0.01
P = 128
NI = N // P          # 4 i-blocks
XW = NI * F          # 256
N_CORES = 8
JG = 4               # Jp granularity (V3 chunk width)

F32 = mybir.dt.float32
BF16 = mybir.dt.bfloat16
FP8 = mybir.dt.float8e4
NPBF16 = np.dtype(ml_dtypes.bfloat16)
NPFP8 = np.dtype(ml_dtypes.float8_e4m3)

_PROG_CACHE = {}


def _splits(Jp):
    """Column partition [V3 | V2 | V1] and block lists."""
    n3 = 8 * max(1, int(round(0.36 * Jp / 8)))
    n1 = 16 * max(0, int(round(0.49 * Jp / 16)))
    while n1 + n3 > Jp - 8 and n1 > 0:
        n1 -= 16
    while n1 + n3 > Jp - 8:
        n3 -= 8
    n2 = Jp - n1 - n3
    v1b = [16] * (n1 // 16) + ([n1 % 16] if n1 % 16 else [])
    v2b = [8] * (n2 // 8) + ([n2 % 8] if n2 % 8 else [])
    v3d = [8] * (n3 // 8)
    return n1, n2, n3, v1b, v2b, v3d


def _build(Jp: int):
    n1, n2, n3, v1b, v2b, v3d = _splits(Jp)
    c2 = n3 + n2          # dense x-correction covers [0, c2)
    # feeder consts (one early DMA): x | xT | idf | xd | adj99 | idp
    CWA = XW + NI * P + F + XW + NI * Jp + P
    # MLP consts (one late DMA): W1 | 0.99W2 | 0.01(W1@W2) | xkT
    CWB = H + 2 * F + Jp

    nc = bacc.Bacc("TRN2", target_bir_lowering=False)
    edge_d = nc.dram_tensor("edge", [P, NI * Jp * F], FP8, kind="ExternalInput")
    csta_d = nc.dram_tensor("csta", [P, CWA], BF16, kind="ExternalInput")
    cstb_d = nc.dram_tensor("cstb", [P, CWB], BF16, kind="ExternalInput")
    cstf_d = nc.dram_tensor("cstf", [P, 2], F32, kind="ExternalInput")
    out_d = nc.dram_tensor("out", [F, Jp], F32, kind="ExternalOutput")

    with tile.TileContext(nc) as tc:
        with tc.tile_pool(name="cpool", bufs=1) as cpool, \
             tc.tile_pool(name="ppool", bufs=1, space="PSUM") as ppool:
            # ---------- static SBUF tiles ----------
            ca_t = cpool.tile([P, CWA], BF16)
            cb_t = cpool.tile([P, CWB], BF16)
            adj_t = cpool.tile([P, NI * Jp], BF16)   # derived 0.01 copy
            cf_t = cpool.tile([P, 2], F32)
            oTs_t = cpool.tile([F, Jp], BF16)

            edge_v = edge_d[:, :].rearrange("p (ib j f) -> p ib j f",
                                            ib=NI, j=Jp)
            o_x = 0
            o_xT = o_x + XW
            o_idf = o_xT + NI * P
            o_xd = o_idf + F
            o_a99 = o_xd + XW
            o_idp = o_a99 + NI * Jp
            x_v = ca_t[:, o_x:o_xT].rearrange("p (ib f) -> p ib f", ib=NI)
            xT_v = ca_t[:F, o_xT:o_idf].rearrange("f (ib i) -> f ib i",
                                                  ib=NI)
            idf_t = ca_t[:F, o_idf:o_xd]
            # x pre-scaled by 0.01/0.99 (dense x-correction rides the 0.99
            # adj copy straight off the DMA — no adj-scale dependency)
            xd_v = ca_t[:, o_xd:o_a99].rearrange("p (ib f) -> p ib f",
                                                 ib=NI)
            adj99_v = ca_t[:, o_a99:o_idp].rearrange("p (ib j) -> p ib j",
                                                     ib=NI)
            idp_t = ca_t[:, o_idp:CWA]
            adj01_v = adj_t[:, :].rearrange("p (ib j) -> p ib j", ib=NI)

            def adjv(s):
                return adj99_v if s else adj01_v

            w1_t = cb_t[:F, 0:H]
            w2a_t = cb_t[:H, H:H + F]
            wlin_t = cb_t[:F, H + F:H + 2 * F]
            xkT_t = cb_t[:F, H + 2 * F:H + 2 * F + Jp]
            b1_t = cf_t[:H, 0:1]
            b2_t = cf_t[:F, 1:2]

            # per-route tiles (j-order: V3 | V2 | V1)
            g = 0
            e3_ts, u3_ts, v3rng = [], [], []
            for w in v3d:
                e3_ts.append(cpool.tile([P, NI * w * F], FP8, name=f"e3_{len(e3_ts)}"))
                u3_ts.append(cpool.tile([P, NI * w * F], BF16, name=f"u3_{len(u3_ts)}"))
                v3rng.append((g, w))
                g += w
            e2_ts, z2_ts, v2rng = [], [], []
            for w in v2b:
                e2_ts.append(cpool.tile([P, NI * w * F], FP8, name=f"e2_{len(e2_ts)}"))
                z2_ts.append(cpool.tile([P, NI * w * F], BF16, name=f"z2_{len(z2_ts)}"))
                v2rng.append((g, w))
                g += w
            z1_ts, u1_ts, v1rng = [], [], []
            for w in v1b:
                z1_ts.append(cpool.tile([P, NI * w * F], BF16, name=f"z1_{len(z1_ts)}"))
                u1_ts.append(cpool.tile([P, NI * w * F], BF16, name=f"u1_{len(u1_ts)}"))
                v1rng.append((g, w))
                g += w
            assert g == Jp

            oT_p = ppool.tile([F, Jp], F32, name="oT")

            def r4(t, w):
                return t[:, :].rearrange("p (ib j f) -> p ib j f",
                                         ib=NI, j=w)

            # ---------- SP: feeder consts first, e DMAs next ----------
            nc.sync.dma_start(out=ca_t[:, :], in_=csta_d[:, :])

            # e DMA interleave: V3 and V2 spread; last DMA is final V3 pair
            ne3, ne2 = len(v3d), len(v2b)
            dma_order = []
            i3 = i2 = 0
            pat = []
            while i3 < ne3 or i2 < ne2:
                if i3 < ne3:
                    pat.append(("v3", i3)); i3 += 1
                if i3 < ne3 - 1 and i2 < ne2:
                    pat.append(("v2", i2)); i2 += 1
                elif i3 >= ne3 and i2 < ne2:
                    pat.append(("v2", i2)); i2 += 1
            dma_order = pat
            for kind, k in dma_order:
                if kind == "v3":
                    g0, w = v3rng[k]
                    nc.sync.dma_start(out=r4(e3_ts[k], w),
                                      in_=edge_v[:, :, g0:g0 + w, :])
                else:
                    g0, w = v2rng[k]
                    nc.sync.dma_start(out=r4(e2_ts[k], w),
                                      in_=edge_v[:, :, g0:g0 + w, :])

            # ---------- DVE: prefill0, adj scale, prefill1 ----------
            if v1rng:
                g0, w = v1rng[0]
                x_b = x_v[:, :, None, :].broadcast_to([P, NI, w, F])
                nc.vector.tensor_copy(r4(z1_ts[0], w), x_b)
            nc.vector.tensor_scalar(
                out=adj_t[:, :], in0=ca_t[:, o_a99:o_idp],
                scalar1=NEG_SLOPE / (1.0 - NEG_SLOPE), scalar2=None,
                op0=mybir.AluOpType.mult)
            for k, (g0, w) in list(enumerate(v1rng))[1:]:
                x_b = x_v[:, :, None, :].broadcast_to([P, NI, w, F])
                nc.vector.tensor_copy(r4(z1_ts[k], w), x_b)

            # ---------- Pool: V1 CCE desc-gens, interleaved with V2
            # casts via issue_cce() calls from the unit loop. Interleaving
            # the gens between casts makes the scheduler's readiness model
            # see the true (late) CCE landing times, so it doesn't hoist
            # V1 relus ahead of ready V2 work on DVE.
            cce_next = [0]

            def issue_cce():
                k = cce_next[0]
                if k < len(v1rng):
                    g0, w = v1rng[k]
                    nc.gpsimd.dma_start(out=r4(z1_ts[k], w),
                                        in_=edge_v[:, :, g0:g0 + w, :],
                                        accum_op=mybir.AluOpType.add)
                    cce_next[0] = k + 1

            issue_cce()

            # ---------- PE: dense x-correction for V2+V3 columns ----------
            # Uses the 0.99 adj copy with host-prescaled x, so it only waits
            # on the cstb + adj DMAs (not the DVE adj-scale).
            if c2 > 0:
                for ib in range(NI):
                    nc.tensor.matmul(oT_p[:, 0:c2], xd_v[:, ib, :],
                                     adj99_v[:, ib, 0:c2],
                                     start=(ib == 0), stop=False)

            # ---------- unit helpers ----------
            CH = JG  # V3 psum chunk width (cols)

            def v3_fill_chunk(k, q):
                """PE: z3 = x + e for chunk q (4 cols) of v3 dma k."""
                g0, w = v3rng[k]
                z3 = ppool.tile([P, NI * CH * F], F32, tag="z3", bufs=2, name="z3")
                z34 = z3[:, :].rearrange("p (ib j f) -> p ib j f",
                                         ib=NI, j=CH)
                e34 = r4(e3_ts[k], w)
                idf_b = idf_t[:, None, :].broadcast_to([F, CH, F])
                for ib in range(NI):
                    nc.tensor.matmul(z34[:, ib, :, :], xT_v[:, ib, :], idf_b,
                                     start=True, stop=False)
                    nc.tensor.matmul(
                        z34[:, ib, :, :], idp_t[:, :],
                        e34[:, ib, q * CH:(q + 1) * CH, :],
                        start=False, stop=True)
                return z3

            def v3_relu_chunk(k, q, z3, eng):
                g0, w = v3rng[k]
                u34 = r4(u3_ts[k], w)
                dst = u34[:, :, q * CH:(q + 1) * CH, :]
                src = z3[:, :].rearrange("p (ib j f) -> p ib j f",
                                         ib=NI, j=CH)
                if eng == "act":
                    nc.scalar.activation(dst, src,
                                         mybir.ActivationFunctionType.Relu)
                else:
                    nc.vector.tensor_scalar(out=dst, in0=src, scalar1=0.0,
                                            scalar2=None,
                                            op0=mybir.AluOpType.max)

            def matvec(slab4, s, g0, w, start, stop):
                for jw in range(w):
                    j = g0 + jw
                    for ib in range(NI):
                        nc.tensor.matmul(
                            oT_p[:, j:j + 1], slab4[:, ib, jw, :],
                            adjv(s)[:, ib, j:j + 1],
                            start=(start and ib == 0),
                            stop=(stop and ib == NI - 1))

            def v2_sf(w):
                return max(1, int(round(w * 0.13)))

            # ---------- processing pipeline ----------
            # unit list in expected arrival order; cce units inserted
            units = []
            nd = len(dma_order)
            nv1 = len(v1rng)
            ins_at = {}
            for k in range(nv1 - 1):
                ins_at.setdefault(min(5 + k, max(0, nd - 1)), []).append(k)
            for idx, (kind, k) in enumerate(dma_order):
                units.append((kind, k))
                for k1 in ins_at.get(idx, []):
                    units.append(("v1", k1))
            if nv1:
                units.append(("v1", nv1 - 1))

            dve_chunks = set()  # v3 chunks relu'd on DVE (rest on ACT)
            v2done = []  # final DVE op per v2 block (ordering anchors)
            chunk_no = 0

            def process(kind, k):
                """elementwise + fills + raw matvecs for a unit."""
                nonlocal chunk_no
                if kind == "v3":
                    g0, w = v3rng[k]
                    nq = w // CH
                    for q in range(nq):
                        z3 = v3_fill_chunk(k, q)
                        eng = "dve" if chunk_no in dve_chunks else "act"
                        v3_relu_chunk(k, q, z3, eng)
                        chunk_no += 1
                    matvec(r4(e3_ts[k], w), 0, g0, w, False, False)
                elif kind == "v2":
                    g0, w = v2rng[k]
                    sf = v2_sf(w)
                    e24 = r4(e2_ts[k], w)
                    z24 = r4(z2_ts[k], w)
                    x_b = x_v[:, :, None, :].broadcast_to([P, NI, w, F])
                    nc.vector.tensor_tensor(
                        out=z24[:, :, 0:sf, :], in0=e24[:, :, 0:sf, :],
                        in1=x_b[:, :, 0:sf, :], op=mybir.AluOpType.add)
                    if sf < w:
                        nc.gpsimd.tensor_copy(z24[:, :, sf:w, :],
                                              e24[:, :, sf:w, :])
                        nc.vector.tensor_tensor(
                            out=z24[:, :, sf:w, :], in0=z24[:, :, sf:w, :],
                            in1=x_b[:, :, sf:w, :], op=mybir.AluOpType.add)
                    matvec(e24, 0, g0, w, False, False)
                    r_i = nc.vector.tensor_scalar(
                        out=z24, in0=z24, scalar1=0.0, scalar2=None,
                        op0=mybir.AluOpType.max)
                    v2done.append(r_i)
                    issue_cce()
                else:  # v1
                    g0, w = v1rng[k]
                    z14 = r4(z1_ts[k], w)
                    # Hard ordering edge: the scheduler's greedy policy uses
                    # an optimistic DMA model and would hoist this relu ahead
                    # of ready V2 work on DVE, stalling it on the CCE sem.
                    r_i = nc.vector.tensor_scalar(
                        out=r4(u1_ts[k], w), in0=z14,
                        scalar1=0.0, scalar2=None, op0=mybir.AluOpType.max)
                    if v2done:
                        anchor = v2done[min(k, len(v2done) - 1)]
                        r_i.ins.add_dependency(
                            anchor.ins.name, mybir.DependencyInfo.NO_SYNC_ONLY)
                    matvec(z14, 0, g0, w, True, False)

            def umv(kind, k):
                if kind == "v3":
                    g0, w = v3rng[k]
                    matvec(r4(u3_ts[k], w), 1, g0, w, False, True)
                elif kind == "v2":
                    g0, w = v2rng[k]
                    matvec(r4(z2_ts[k], w), 1, g0, w, False, True)
                else:
                    g0, w = v1rng[k]
                    matvec(r4(u1_ts[k], w), 1, g0, w, False, True)

            def mlp(c0, c1_, last):
                CW = c1_ - c0
                nc.vector.tensor_tensor(
                    out=oTs_t[:, c0:c1_], in0=oT_p[:, c0:c1_],
                    in1=xkT_t[:, c0:c1_], op=mybir.AluOpType.add)
                h_p = ppool.tile([H, CW], F32, tag="hp", bufs=2,
                                 padded_shape=[H, Jp])
                nc.tensor.matmul(h_p[:, :], w1_t, oTs_t[:, c0:c1_],
                                 start=True, stop=True)
                y_p = ppool.tile([F, CW], F32, tag="yp", bufs=1,
                                 padded_shape=[F, Jp])
                nc.tensor.matmul(y_p[:, :], wlin_t, oTs_t[:, c0:c1_],
                                 start=True, stop=False)
                h_s = cpool.tile([H, CW], BF16,
                                 name=f"hs{c0}")
                nc.scalar.activation(h_s[:, :], h_p[:, :],
                                     mybir.ActivationFunctionType.Relu,
                                     bias=b1_t)
                nc.tensor.matmul(y_p[:, :], w2a_t, h_s[:, :],
                                 start=False, stop=True)
                y_s = cpool.tile([F, CW], F32, name=f"ys{c0}")
                if last:
                    nc.vector.tensor_tensor(
                        out=y_s[:, :], in0=y_p[:, :],
                        in1=b2_t.broadcast_to([F, CW]),
                        op=mybir.AluOpType.add)
                else:
                    nc.scalar.activation(
                        y_s[:, :], y_p[:, :],
                        mybir.ActivationFunctionType.Identity, bias=b2_t)
                nc.sync.dma_start(out=out_d[:, c0:c1_], in_=y_s[:, :])

            while cce_next[0] < len(v1rng):
                issue_cce()

            # MLP consts + biases ride after the e-stream on SP
            nc.sync.dma_start(out=cb_t[:, :], in_=cstb_d[:, :])
            nc.sync.dma_start(out=cf_t[:, :], in_=cstf_d[:, :])

            SKEW = 4
            NU = len(units)
            cut = Jp - units[-1][1] if False else Jp - (
                v1rng[-1][1] if v1rng else v3rng[-1][1])  # last unit -> MLP-B
            last_unit = units[-1]
            for i in range(NU):
                process(*units[i])
                if i >= SKEW and units[i - SKEW] != last_unit:
                    umv(*units[i - SKEW])
            for i in range(max(0, NU - SKEW), NU):
                if units[i] != last_unit:
                    umv(*units[i])
            if cut > 0:
                mlp(0, cut, False)
            with tc.high_priority():
                umv(*last_unit)
                mlp(cut, Jp, True)

    nc.compile()
    return nc


def _get_prog(Jp: int):
    if Jp not in _PROG_CACHE:
        _PROG_CACHE[Jp] = _build(Jp)
    return _PROG_CACHE[Jp]


def kernel(x, adj, edge_attr, mask, W1, b1, W2, b2):
    x = np.ascontiguousarray(np.asarray(x, dtype=np.float32))
    adj = np.ascontiguousarray(np.asarray(adj, dtype=np.float32))
    edge_attr = np.ascontiguousarray(np.asarray(edge_attr, dtype=np.float32))
    mask = np.asarray(mask)
    W1 = np.asarray(W1, dtype=np.float32)
    b1 = np.asarray(b1, dtype=np.float32)
    W2 = np.asarray(W2, dtype=np.float32)
    b2 = np.asarray(b2, dtype=np.float32)

    core_jj = []
    for b in range(B):
        jj = np.flatnonzero(mask[b])
        core_jj.append(jj[0::2])
        core_jj.append(jj[1::2])
    maxJ = max((len(jj) for jj in core_jj), default=1)
    Jp = max(16, ((maxJ + JG - 1) // JG) * JG)

    nc = _get_prog(Jp)

    CWA = XW + NI * P + F + XW + NI * Jp + P
    CWB = H + 2 * F + Jp
    o_x = 0
    o_xT = o_x + XW
    o_idf = o_xT + NI * P
    o_xd = o_idf + F
    o_a99 = o_xd + XW
    o_idp = o_a99 + NI * Jp
    in_maps = []
    for c, jj in enumerate(core_jj):
        b = c // 2
        J = len(jj)
        edge_c = np.zeros((N, Jp, F), np.float32)
        if J:
            edge_c[:, :J] = edge_attr[b][:, jj, :]
        # [P, NI, Jp, F] layout, fp8
        edge_r = np.ascontiguousarray(
            edge_c.reshape(NI, P, Jp, F).transpose(1, 0, 2, 3)
        ).reshape(P, NI * Jp * F).astype(NPFP8)
        adj_c = np.zeros((N, Jp), np.float32)
        if J:
            adj_c[:, :J] = adj[b][:, jj]
        adj_ibpj = adj_c.reshape(NI, P, Jp).transpose(1, 0, 2)
        x_r = x[b].reshape(NI, P, F).transpose(1, 0, 2).reshape(P, NI * F)
        csta = np.zeros((P, CWA), NPBF16)
        csta[:, o_x:o_xT] = x_r.astype(NPBF16)
        csta[:F, o_xT:o_idf] = x[b].T.astype(NPBF16)
        csta[:F, o_idf:o_xd] = np.eye(F, dtype=np.float32).astype(NPBF16)
        csta[:, o_xd:o_a99] = (
            (NEG_SLOPE / (1.0 - NEG_SLOPE)) * x_r).astype(NPBF16)
        csta[:, o_a99:o_idp] = ((1.0 - NEG_SLOPE) * adj_ibpj).reshape(
            P, NI * Jp).astype(NPBF16)
        csta[:, o_idp:CWA] = np.eye(P, dtype=np.float32).astype(NPBF16)
        cstb = np.zeros((P, CWB), NPBF16)
        cstb[:F, 0:H] = W1.astype(NPBF16)
        cstb[:H, H:H + F] = ((1.0 - NEG_SLOPE) * W2).astype(NPBF16)
        cstb[:F, H + F:H + 2 * F] = (NEG_SLOPE * (W1 @ W2)).astype(NPBF16)
        if J:
            cstb[:F, H + 2 * F:H + 2 * F + J] = x[b][jj].T.astype(NPBF16)
        cstf = np.zeros((P, 2), np.float32)
        cstf[:H, 0] = b1
        cstf[:F, 1] = NEG_SLOPE * (b1 @ W2) + b2
        in_maps.append({
            "edge": edge_r, "csta": csta, "cstb": cstb, "cstf": cstf,
        })

    res = run_bass_kernel_spmd(nc, in_maps, list(range(N_CORES)))

    out = np.zeros((B, N, F), np.float32)
    for c, jj in enumerate(core_jj):
        b = c // 2
        if len(jj):
            out[b][jj] = res.results[c]["out"][:, :len(jj)].T
    return out


# revision 44
# speedup vs baseline: 1.0068x; 1.0068x over previous
"""DenseGINEConv on 8 TRN2 NeuronCores (Bass/Tile) — tri-route fp8 edition.

Reference computation (B=4, N=512, F=64, H=128):
    msg  = leaky_relu(adj[b,i,j] * (x[b,i,f] + edge_attr[b,i,j,f]), 0.01)
    agg  = sum_i msg                         # (B, N, F) indexed by destination j
    out  = x + agg
    h    = leaky_relu(out @ W1 + b1) @ W2 + b2
    res  = where(mask[b,j], h, 0)

Key facts:
  * adj >= 0, so lrelu(adj*z) = adj*lrelu(z) = 0.01*adj*z + 0.99*adj*relu(z).
  * Kept destination nodes only (host j-compaction); core c = 2*b + h.
  * Edge ships as fp8e4 (rel-err ~8e-3 vs the 2e-2 gate). The cost model
    prices a DMA at its OUTPUT bytes, so a cast-up CCE DMA is priced bf16.
    Hence three routes for z = x + e, u = relu(z), each burning a different
    scarce resource:
      V1 (CCE):   bf16 z tile prefilled with broadcast x (DVE 4x copy), SWDGE
                  CCE add casts fp8->bf16 in the DMA (DMA pays 2B/elem, DVE
                  pays 0.26ns/elem for prefill + relu).
      V2 (engine): plain fp8 e tile over HWDGE (1B/elem). Pool casts most
                  columns fp8->bf16, DVE adds broadcast x (2x) and a few
                  fully-fused columns (1x), relu via DVE tensor_scalar (4x).
      V3 (PE):    plain fp8 e tile. PE accumulates z in PSUM: an x-fill
                  matmul (xT stationary x broadcast-identity moving) plus an
                  e-copy matmul (fp8 identity stationary x e moving); ACT
                  (mostly) applies relu PSUM->SBUF bf16.
    The raw 0.01*z stream for V2/V3 never materializes z: sum_i adj*z =
    (x^T @ 0.01adj) dense matmuls + matvecs directly on the fp8 e tiles
    (mixed fp8 stationary x bf16 moving works).
  * Aggregation: per destination j, single-column matmuls accumulate
    oT[:, j] into one [F, Jp] PSUM tile (raw stream vs 0.01*adj, relu
    stream vs 0.99*adj; only the 0.99 copy ships, DVE derives 0.01).
  * MLP tail in two column chunks: y = relu(o@W1+b1) @ 0.99W2
    + o @ 0.01(W1@W2) + b2', b2' = 0.01*(b1@W2) + b2 host-folded.
"""
import numpy as np
import ml_dtypes

import concourse.bacc as bacc
import concourse.mybir as mybir
import concourse.tile as tile
from concourse.bass_utils import run_bass_kernel_spmd

B, N, F, H = 4, 512, 64, 128
NEG_SLOPE = 0.01
P = 128
NI = N // P          # 4 i-blocks
XW = NI * F          # 256
N_CORES = 8
JG = 4               # Jp granularity (V3 chunk width)

F32 = mybir.dt.float32
BF16 = mybir.dt.bfloat16
FP8 = mybir.dt.float8e4
NPBF16 = np.dtype(ml_dtypes.bfloat16)
NPFP8 = np.dtype(ml_dtypes.float8_e4m3)

_PROG_CACHE = {}


def _splits(Jp):
    """Column partition [V3 | V2 | V1] and block lists."""
    n3 = 8 * max(1, int(round(0.36 * Jp / 8)))
    n1 = 16 * max(0, int(round(0.49 * Jp / 16)))
    while n1 + n3 > Jp - 8 and n1 > 0:
        n1 -= 8
    while n1 + n3 > Jp - 8:
        n3 -= 8
    n2 = Jp - n1 - n3
    if n1 >= 48:
        # taper: the last CCE blocks small so the tail relu + MLP-B chain
        # (gated by the final CCE transfers) starts earlier
        h1 = ((n1 - 16) // 2 + 7) // 8 * 8
        v1b = [h1, (n1 - 16) - h1, 8, 8]
    else:
        v1b = [16] * (n1 // 16) + ([n1 % 16] if n1 % 16 else [])
    v2b = [8] * (n2 // 8) + ([n2 % 8] if n2 % 8 else [])
    v3d = [8] * (n3 // 8)
    return n1, n2, n3, v1b, v2b, v3d


def _build(Jp: int):
    n1, n2, n3, v1b, v2b, v3d = _splits(Jp)
    c2 = n3 + n2          # dense x-correction covers [0, c2)
    # feeder consts (one early DMA): x | xT | xd | adj99 | idp
    CWA = XW + NI * P + XW + NI * Jp + P
    # MLP consts (one late DMA): W1 | 0.99W2 | 0.01(W1@W2) | xkT
    CWB = H + 2 * F + Jp

    nc = bacc.Bacc("TRN2", target_bir_lowering=False)
    edge_d = nc.dram_tensor("edge", [P, NI * Jp * F], FP8, kind="ExternalInput")
    csta_d = nc.dram_tensor("csta", [P, CWA], BF16, kind="ExternalInput")
    cstb_d = nc.dram_tensor("cstb", [P, CWB], BF16, kind="ExternalInput")
    cstf_d = nc.dram_tensor("cstf", [P, 2], F32, kind="ExternalInput")
    out_d = nc.dram_tensor("out", [F, Jp], F32, kind="ExternalOutput")

    with tile.TileContext(nc) as tc:
        with tc.tile_pool(name="cpool", bufs=1) as cpool, \
             tc.tile_pool(name="ppool", bufs=1, space="PSUM") as ppool:
            # ---------- static SBUF tiles ----------
            ca_t = cpool.tile([P, CWA], BF16)
            cb_t = cpool.tile([P, CWB], BF16)
            adj_t = cpool.tile([P, NI * Jp], BF16)   # derived 0.01 copy
            cf_t = cpool.tile([P, 2], F32)
            oTs_t = cpool.tile([F, Jp], BF16)

            edge_v = edge_d[:, :].rearrange("p (ib j f) -> p ib j f",
                                            ib=NI, j=Jp)
            o_x = 0
            o_xT = o_x + XW
            o_xd = o_xT + NI * P
            o_a99 = o_xd + XW
            o_idp = o_a99 + NI * Jp
            x_v = ca_t[:, o_x:o_xT].rearrange("p (ib f) -> p ib f", ib=NI)
            xT_v = ca_t[:F, o_xT:o_xd].rearrange("f (ib i) -> f ib i",
                                                 ib=NI)
            # x pre-scaled by 0.01/0.99 (dense x-correction rides the 0.99
            # adj copy straight off the DMA — no adj-scale dependency)
            xd_v = ca_t[:, o_xd:o_a99].rearrange("p (ib f) -> p ib f",
                                                 ib=NI)
            adj99_v = ca_t[:, o_a99:o_idp].rearrange("p (ib j) -> p ib j",
                                                     ib=NI)
            idp_t = ca_t[:, o_idp:CWA]
            idf_t = idp_t[:F, 0:F]   # top-left 64x64 of the identity
            adj01_v = adj_t[:, :].rearrange("p (ib j) -> p ib j", ib=NI)

            def adjv(s):
                return adj99_v if s else adj01_v

            w1_t = cb_t[:F, 0:H]
            w2a_t = cb_t[:H, H:H + F]
            wlin_t = cb_t[:F, H + F:H + 2 * F]
            xkT_t = cb_t[:F, H + 2 * F:H + 2 * F + Jp]
            b1_t = cf_t[:H, 0:1]
            b2_t = cf_t[:F, 1:2]

            # per-route tiles (j-order: V3 | V2 | V1)
            g = 0
            e3_ts, u3_ts, v3rng = [], [], []
            for w in v3d:
                e3_ts.append(cpool.tile([P, NI * w * F], FP8, name=f"e3_{len(e3_ts)}"))
                u3_ts.append(cpool.tile([P, NI * w * F], BF16, name=f"u3_{len(u3_ts)}"))
                v3rng.append((g, w))
                g += w
            e2_ts, z2_ts, v2rng = [], [], []
            for w in v2b:
                e2_ts.append(cpool.tile([P, NI * w * F], FP8, name=f"e2_{len(e2_ts)}"))
                z2_ts.append(cpool.tile([P, NI * w * F], BF16, name=f"z2_{len(z2_ts)}"))
                v2rng.append((g, w))
                g += w
            z1_ts, u1_ts, v1rng = [], [], []
            for w in v1b:
                z1_ts.append(cpool.tile([P, NI * w * F], BF16, name=f"z1_{len(z1_ts)}"))
                u1_ts.append(cpool.tile([P, NI * w * F], BF16, name=f"u1_{len(u1_ts)}"))
                v1rng.append((g, w))
                g += w
            assert g == Jp

            oT_p = ppool.tile([F, Jp], F32, name="oT")

            def r4(t, w):
                return t[:, :].rearrange("p (ib j f) -> p ib j f",
                                         ib=NI, j=w)

            # ---------- SP: feeder consts first, e DMAs next ----------
            nc.sync.dma_start(out=ca_t[:, :], in_=csta_d[:, :])

            # e DMA interleave: V3 and V2 spread; last DMA is final V3 pair
            ne3, ne2 = len(v3d), len(v2b)
            dma_order = []
            i3 = i2 = 0
            pat = []
            while i3 < ne3 or i2 < ne2:
                if i3 < ne3:
                    pat.append(("v3", i3)); i3 += 1
                if i3 < ne3 - 1 and i2 < ne2:
                    pat.append(("v2", i2)); i2 += 1
                elif i3 >= ne3 and i2 < ne2:
                    pat.append(("v2", i2)); i2 += 1
            dma_order = pat
            for kind, k in dma_order:
                if kind == "v3":
                    g0, w = v3rng[k]
                    nc.sync.dma_start(out=r4(e3_ts[k], w),
                                      in_=edge_v[:, :, g0:g0 + w, :])
                else:
                    g0, w = v2rng[k]
                    nc.sync.dma_start(out=r4(e2_ts[k], w),
                                      in_=edge_v[:, :, g0:g0 + w, :])

            # ---------- DVE: prefill0, adj scale, prefill1 ----------
            if v1rng:
                g0, w = v1rng[0]
                x_b = x_v[:, :, None, :].broadcast_to([P, NI, w, F])
                nc.vector.tensor_copy(r4(z1_ts[0], w), x_b)
            nc.vector.tensor_scalar(
                out=adj_t[:, :], in0=ca_t[:, o_a99:o_idp],
                scalar1=NEG_SLOPE / (1.0 - NEG_SLOPE), scalar2=None,
                op0=mybir.AluOpType.mult)
            for k, (g0, w) in list(enumerate(v1rng))[1:]:
                x_b = x_v[:, :, None, :].broadcast_to([P, NI, w, F])
                nc.vector.tensor_copy(r4(z1_ts[k], w), x_b)

            # ---------- Pool: V1 CCE desc-gens, interleaved with V2
            # casts via issue_cce() calls from the unit loop. Interleaving
            # the gens between casts makes the scheduler's readiness model
            # see the true (late) CCE landing times, so it doesn't hoist
            # V1 relus ahead of ready V2 work on DVE.
            cce_next = [0]

            def issue_cce():
                k = cce_next[0]
                if k < len(v1rng):
                    g0, w = v1rng[k]
                    nc.gpsimd.dma_start(out=r4(z1_ts[k], w),
                                        in_=edge_v[:, :, g0:g0 + w, :],
                                        accum_op=mybir.AluOpType.add)
                    cce_next[0] = k + 1

            issue_cce()

            # ---------- PE: dense x-correction for V2+V3 columns ----------
            # Uses the 0.99 adj copy with host-prescaled x, so it only waits
            # on the cstb + adj DMAs (not the DVE adj-scale).
            if c2 > 0:
                for ib in range(NI):
                    nc.tensor.matmul(oT_p[:, 0:c2], xd_v[:, ib, :],
                                     adj99_v[:, ib, 0:c2],
                                     start=(ib == 0), stop=False)

            # ---------- unit helpers ----------
            CH = JG  # V3 psum chunk width (cols)

            def v3_fill_chunk(k, q):
                """PE: z3 = x + e for chunk q (4 cols) of v3 dma k."""
                g0, w = v3rng[k]
                z3 = ppool.tile([P, NI * CH * F], F32, tag="z3", bufs=2, name="z3")
                z34 = z3[:, :].rearrange("p (ib j f) -> p ib j f",
                                         ib=NI, j=CH)
                e34 = r4(e3_ts[k], w)
                idf_b = idf_t[:, None, :].broadcast_to([F, CH, F])
                for ib in range(NI):
                    nc.tensor.matmul(z34[:, ib, :, :], xT_v[:, ib, :], idf_b,
                                     start=True, stop=False)
                    nc.tensor.matmul(
                        z34[:, ib, :, :], idp_t[:, :],
                        e34[:, ib, q * CH:(q + 1) * CH, :],
                        start=False, stop=True)
                return z3

            def v3_relu_chunk(k, q, z3, eng):
                g0, w = v3rng[k]
                u34 = r4(u3_ts[k], w)
                dst = u34[:, :, q * CH:(q + 1) * CH, :]
                src = z3[:, :].rearrange("p (ib j f) -> p ib j f",
                                         ib=NI, j=CH)
                if eng == "act":
                    nc.scalar.activation(dst, src,
                                         mybir.ActivationFunctionType.Relu)
                else:
                    nc.vector.tensor_scalar(out=dst, in0=src, scalar1=0.0,
                                            scalar2=None,
                                            op0=mybir.AluOpType.max)

            def matvec(slab4, s, g0, w, start, stop):
                for jw in range(w):
                    j = g0 + jw
                    for ib in range(NI):
                        nc.tensor.matmul(
                            oT_p[:, j:j + 1], slab4[:, ib, jw, :],
                            adjv(s)[:, ib, j:j + 1],
                            start=(start and ib == 0),
                            stop=(stop and ib == NI - 1))

            def v2_sf(w):
                return max(1, int(round(w * 0.13)))

            # ---------- processing pipeline ----------
            # unit list in expected arrival order; cce units inserted
            units = []
            nd = len(dma_order)
            nv1 = len(v1rng)
            ins_at = {}
            for k in range(nv1 - 1):
                ins_at.setdefault(min(5 + k, max(0, nd - 1)), []).append(k)
            for idx, (kind, k) in enumerate(dma_order):
                units.append((kind, k))
                for k1 in ins_at.get(idx, []):
                    units.append(("v1", k1))
            if nv1:
                units.append(("v1", nv1 - 1))

            dve_chunks = set()  # v3 chunks relu'd on DVE (rest on ACT)
            v2done = []  # final DVE op per v2 block (ordering anchors)
            chunk_no = 0

            def process(kind, k):
                """elementwise + fills + raw matvecs for a unit."""
                nonlocal chunk_no
                if kind == "v3":
                    g0, w = v3rng[k]
                    nq = w // CH
                    for q in range(nq):
                        z3 = v3_fill_chunk(k, q)
                        eng = "dve" if chunk_no in dve_chunks else "act"
                        v3_relu_chunk(k, q, z3, eng)
                        chunk_no += 1
                    matvec(r4(e3_ts[k], w), 0, g0, w, False, False)
                elif kind == "v2":
                    g0, w = v2rng[k]
                    sf = v2_sf(w)
                    e24 = r4(e2_ts[k], w)
                    z24 = r4(z2_ts[k], w)
                    x_b = x_v[:, :, None, :].broadcast_to([P, NI, w, F])
                    nc.vector.tensor_tensor(
                        out=z24[:, :, 0:sf, :], in0=e24[:, :, 0:sf, :],
                        in1=x_b[:, :, 0:sf, :], op=mybir.AluOpType.add)
                    if sf < w:
                        nc.gpsimd.tensor_copy(z24[:, :, sf:w, :],
                                              e24[:, :, sf:w, :])
                        nc.vector.tensor_tensor(
                            out=z24[:, :, sf:w, :], in0=z24[:, :, sf:w, :],
                            in1=x_b[:, :, sf:w, :], op=mybir.AluOpType.add)
                    matvec(e24, 0, g0, w, False, False)
                    r_i = nc.vector.tensor_scalar(
                        out=z24, in0=z24, scalar1=0.0, scalar2=None,
                        op0=mybir.AluOpType.max)
                    v2done.append(r_i)
                    issue_cce()
                else:  # v1
                    g0, w = v1rng[k]
                    z14 = r4(z1_ts[k], w)
                    # Hard ordering edge: the scheduler's greedy policy uses
                    # an optimistic DMA model and would hoist this relu ahead
                    # of ready V2 work on DVE, stalling it on the CCE sem.
                    r_i = nc.vector.tensor_scalar(
                        out=r4(u1_ts[k], w), in0=z14,
                        scalar1=0.0, scalar2=None, op0=mybir.AluOpType.max)
                    if v2done:
                        anchor = v2done[min(k, len(v2done) - 1)]
                        r_i.ins.add_dependency(
                            anchor.ins.name, mybir.DependencyInfo.NO_SYNC_ONLY)
                    matvec(z14, 0, g0, w, True, False)

            def umv(kind, k):
                if kind == "v3":
                    g0, w = v3rng[k]
                    matvec(r4(u3_ts[k], w), 1, g0, w, False, True)
                elif kind == "v2":
                    g0, w = v2rng[k]
                    matvec(r4(z2_ts[k], w), 1, g0, w, False, True)
                else:
                    g0, w = v1rng[k]
                    matvec(r4(u1_ts[k], w), 1, g0, w, False, True)

            def mlp(c0, c1_, last):
                CW = c1_ - c0
                nc.vector.tensor_tensor(
                    out=oTs_t[:, c0:c1_], in0=oT_p[:, c0:c1_],
                    in1=xkT_t[:, c0:c1_], op=mybir.AluOpType.add)
                h_p = ppool.tile([H, CW], F32, tag="hp", bufs=2,
                                 padded_shape=[H, Jp])
                nc.tensor.matmul(h_p[:, :], w1_t, oTs_t[:, c0:c1_],
                                 start=True, stop=True)
                y_p = ppool.tile([F, CW], F32, tag="yp", bufs=1,
                                 padded_shape=[F, Jp])
                nc.tensor.matmul(y_p[:, :], wlin_t, oTs_t[:, c0:c1_],
                                 start=True, stop=False)
                h_s = cpool.tile([H, CW], BF16,
                                 name=f"hs{c0}")
                nc.scalar.activation(h_s[:, :], h_p[:, :],
                                     mybir.ActivationFunctionType.Relu,
                                     bias=b1_t)
                nc.tensor.matmul(y_p[:, :], w2a_t, h_s[:, :],
                                 start=False, stop=True)
                y_s = cpool.tile([F, CW], F32, name=f"ys{c0}")
                if last:
                    nc.vector.tensor_tensor(
                        out=y_s[:, :], in0=y_p[:, :],
                        in1=b2_t.broadcast_to([F, CW]),
                        op=mybir.AluOpType.add)
                else:
                    nc.scalar.activation(
                        y_s[:, :], y_p[:, :],
                        mybir.ActivationFunctionType.Identity, bias=b2_t)
                nc.sync.dma_start(out=out_d[:, c0:c1_], in_=y_s[:, :])

            while cce_next[0] < len(v1rng):
                issue_cce()

            # MLP consts + biases ride after the e-stream on SP
            nc.sync.dma_start(out=cb_t[:, :], in_=cstb_d[:, :])
            nc.sync.dma_start(out=cf_t[:, :], in_=cstf_d[:, :])

            SKEW = 4
            NU = len(units)
            cut = Jp - (v1rng[-1][1] if v1rng else v3rng[-1][1])
            last_unit = units[-1]
            for i in range(NU):
                process(*units[i])
                if i >= SKEW and units[i - SKEW] != last_unit:
                    umv(*units[i - SKEW])
            for i in range(max(0, NU - SKEW), NU):
                if units[i] != last_unit:
                    umv(*units[i])
            if cut > 0:
                mlp(0, cut, False)
            with tc.high_priority():
                umv(*last_unit)
                mlp(cut, Jp, True)

    nc.compile()
    return nc


def _get_prog(Jp: int):
    if Jp not in _PROG_CACHE:
        _PROG_CACHE[Jp] = _build(Jp)
    return _PROG_CACHE[Jp]


def kernel(x, adj, edge_attr, mask, W1, b1, W2, b2):
    x = np.ascontiguousarray(np.asarray(x, dtype=np.float32))
    adj = np.ascontiguousarray(np.asarray(adj, dtype=np.float32))
    edge_attr = np.ascontiguousarray(np.asarray(edge_attr, dtype=np.float32))
    mask = np.asarray(mask)
    W1 = np.asarray(W1, dtype=np.float32)
    b1 = np.asarray(b1, dtype=np.float32)
    W2 = np.asarray(W2, dtype=np.float32)
    b2 = np.asarray(b2, dtype=np.float32)

    core_jj = []
    for b in range(B):
        jj = np.flatnonzero(mask[b])
        core_jj.append(jj[0::2])
        core_jj.append(jj[1::2])
    maxJ = max((len(jj) for jj in core_jj), default=1)
    Jp = max(16, ((maxJ + JG - 1) // JG) * JG)

    nc = _get_prog(Jp)

    CWA = XW + NI * P + XW + NI * Jp + P
    CWB = H + 2 * F + Jp
    o_x = 0
    o_xT = o_x + XW
    o_xd = o_xT + NI * P
    o_a99 = o_xd + XW
    o_idp = o_a99 + NI * Jp
    in_maps = []
    for c, jj in enumerate(core_jj):
        b = c // 2
        J = len(jj)
        edge_c = np.zeros((N, Jp, F), np.float32)
        if J:
            edge_c[:, :J] = edge_attr[b][:, jj, :]
        # [P, NI, Jp, F] layout, fp8
        edge_r = np.ascontiguousarray(
            edge_c.reshape(NI, P, Jp, F).transpose(1, 0, 2, 3)
        ).reshape(P, NI * Jp * F).astype(NPFP8)
        adj_c = np.zeros((N, Jp), np.float32)
        if J:
            adj_c[:, :J] = adj[b][:, jj]
        adj_ibpj = adj_c.reshape(NI, P, Jp).transpose(1, 0, 2)
        x_r = x[b].reshape(NI, P, F).transpose(1, 0, 2).reshape(P, NI * F)
        csta = np.zeros((P, CWA), NPBF16)
        csta[:, o_x:o_xT] = x_r.astype(NPBF16)
        csta[:F, o_xT:o_xd] = x[b].T.astype(NPBF16)
        csta[:, o_xd:o_a99] = (
            (NEG_SLOPE / (1.0 - NEG_SLOPE)) * x_r).astype(NPBF16)
        csta[:, o_a99:o_idp] = ((1.0 - NEG_SLOPE) * adj_ibpj).reshape(
            P, NI * Jp).astype(NPBF16)
        csta[:, o_idp:CWA] = np.eye(P, dtype=np.float32).astype(NPBF16)
        cstb = np.zeros((P, CWB), NPBF16)
        cstb[:F, 0:H] = W1.astype(NPBF16)
        cstb[:H, H:H + F] = ((1.0 - NEG_SLOPE) * W2).astype(NPBF16)
        cstb[:F, H + F:H + 2 * F] = (NEG_SLOPE * (W1 @ W2)).astype(NPBF16)
        if J:
            cstb[:F, H + 2 * F:H + 2 * F + J] = x[b][jj].T.astype(NPBF16)
        cstf = np.zeros((P, 2), np.float32)
        cstf[:H, 0] = b1
        cstf[:F, 1] = NEG_SLOPE * (b1 @ W2) + b2
        in_maps.append({
            "edge": edge_r, "csta": csta, "cstb": cstb, "cstf": cstf,
        })

    res = run_bass_kernel_spmd(nc, in_maps, list(range(N_CORES)))

    out = np.zeros((B, N, F), np.float32)
    for c, jj in enumerate(core_jj):
        b = c // 2
        if len(jj):
            out[b][jj] = res.results[c]["out"][:, :len(jj)].T
    return out
